# revision 14
# baseline (speedup 1.0000x reference)
"""MLS rigid deformation (Schaefer et al.) dense remap grid on 8 trn2 cores.

Math: per pixel v=(x,y), weights w_n = 1/(|pi_n - v|^2 + 1e-9). The 2x2 MLS
similarity matrix is a scaled rotation, so the whole reduction collapses to 7
weighted sums per pixel:
  sw, Spx, Spy, Sqx, Sqy, Spq = sum w*pi.qi, Sx = sum w*(qix*piy - qiy*pix)
with
  ps = (Spx,Spy)/sw, qs = (Sqx,Sqy)/sw
  P = Spq - (Spx*Sqx + Spy*Sqy)/sw
  Q = Sx  - (Sqx*Spy - Sqy*Spx)/sw
  vp = v - ps; frv = (P*vpx + Q*vpy, -Q*vpx + P*vpy)
  out = |vp| * frv/(|frv|+1e-10) + qs

Sharding: W (x) dimension across 8 cores, 96 columns each.

Per-core device pipeline (96 "units", unit u = (x-pair p=u//2, y-half h=u%2),
each unit = 2 x-columns * 384 y = 768 pixels):
  1. ACT Reciprocal per unit: t = 1/(sqy_h * icx_u + 1) where icx_u[pt] =
     1/((x-pix)^2+eps) is a per-partition scale AP and sqy_h[pt,y] =
     (y-piy)^2. Then w = t * icx_u, with the icx_u factor folded into the
     per-unit coefficient matrix c2u (so no separate multiply of t).
  2. DVE per unit: c2u [128,14] = c2 * icx_u (tiny).
  3. pixel-major sums matmul (fp32 exact, N=14): per 128-col chunk c:
     out[128(y-chunk), 14] = t_chunk.T @ c2u, packed into PSUM bank [128,504].
  4. ACT copy bank -> Ebuf [128, 4032] (col = (3u+c)*14 + 7e + s).
  5. Elementwise epilogue (DVE + ACT sqrt + exact DVE recip) in 2 passes
     (e = x parity), writing interleaved out_xy [128, 1152] in f16.
  6. 2 output DMAs -> out [768, 192] f16 (y-major, (x_loc, comp) contiguous).

Host/dispatch: the expensive part of the baseline was not the device but the
axon tunnel: per-call jit re-trace + re-compile, a 25.6MB lhsT re-upload and
f32 output download. Here the executable is AOT-compiled once and cached,
coordinate-grid constants live on device across calls, per-call uploads are
~1.2MB total (icx/sqy/c2), and the output returns as f16 (2.4MB).
"""

import numpy as np

H = 768
W = 768
N = 64
NCORES = 8
WLOC = W // NCORES        # 96 x-columns per core
NU = WLOC                 # 96 units (pair, half)
NCH = 3 * NU              # 288 chunks of 128 pixel-rows
YH = 384                  # y half height
EPS_D2 = 1e-9
EPS_FRV = 1e-10
CTR = 384.0               # coordinate centering for coefficient magnitudes

_CACHE = {}


def _build_nc():
    import concourse.bass as bass
    import concourse.mybir as mybir
    from concourse.tile import TileContext

    F32 = mybir.dt.float32
    F16 = mybir.dt.float16

    def act_recip(nc, out, in_, scale_ap=None, bias=0.0):
        # ACT table reciprocal (~2.4e-4 rel err): fine for the MLS weights,
        # whose consistent perturbation cancels in the weighted averages.
        # out = 1/(in*scale + bias); scale may be a per-partition [128,1] AP.
        ins = [nc.scalar.lower_ap(in_),
               mybir.ImmediateValue(dtype=mybir.dt.float32, value=bias)]
        if scale_ap is None:
            ins.append(mybir.ImmediateValue(dtype=mybir.dt.float32, value=1.0))
        else:
            ins.append(nc.scalar.lower_ap(scale_ap))
        ins.append(mybir.ImmediateValue(dtype=mybir.dt.float32, value=0.0))
        return nc.scalar.add_instruction(mybir.InstActivation(
            name=nc.get_next_instruction_name(),
            func=mybir.ActivationFunctionType.Reciprocal,
            ins=ins, outs=[nc.scalar.lower_ap(out)]))

    nc = bass.Bass()
    # pk packs all per-call (pi/qi-dependent) data into one small upload:
    # cols 0:48 icx (1/((x-pix)^2+eps), pair-indexed), col 48 piy (both parity
    # halves), cols 49:63 the c2 coefficient matrix.
    pkd = nc.dram_tensor("pk", [128, 63], F32, kind="ExternalInput")
    yrd = nc.dram_tensor("yramp", [128, H], F32, kind="ExternalInput")
    xg0d = nc.dram_tensor("xg0", [128, NCH], F32, kind="ExternalInput")
    xg1d = nc.dram_tensor("xg1", [128, NCH], F32, kind="ExternalInput")
    ygd = nc.dram_tensor("yg", [128, NCH], F32, kind="ExternalInput")
    outd = nc.dram_tensor("out", [H, 2 * WLOC], F16, kind="ExternalOutput")

    AL = mybir.AluOpType

    with TileContext(nc) as tc:
        with (
            tc.tile_pool(name="const", bufs=1) as cpool,
            tc.tile_pool(name="w", bufs=3) as wpool,
            tc.tile_pool(name="c2u", bufs=3) as upool,
            tc.tile_pool(name="ebuf", bufs=1) as epool,
            tc.tile_pool(name="epi", bufs=1) as tpool,
            tc.tile_pool(name="pssum", bufs=2, space="PSUM") as pssum,
        ):
            pk = cpool.tile([128, 63], F32, tag="pk")
            nc.sync.dma_start(out=pk[:], in_=pkd[:])
            yramp = cpool.tile([128, H], F32, tag="yramp")
            nc.sync.dma_start(out=yramp[:], in_=yrd[:])
            xg = [cpool.tile([128, NCH], F32, tag="xg0", name="xg0"),
                  cpool.tile([128, NCH], F32, tag="xg1", name="xg1")]
            nc.sync.dma_start(out=xg[0][:], in_=xg0d[:])
            nc.sync.dma_start(out=xg[1][:], in_=xg1d[:])
            yg = cpool.tile([128, NCH], F32, tag="yg")
            nc.sync.dma_start(out=yg[:], in_=ygd[:])
            c2 = pk[:, 49:63]

            # sqy_h[pt, y] = (y_global - piy)^2 computed on device (exact f32
            # sub + mult): saves shipping the 1.5MB table per call.
            sq = [cpool.tile([128, YH], F32, tag="sq0", name="sq0"),
                  cpool.tile([128, YH], F32, tag="sq1", name="sq1")]
            ydiff = cpool.tile([128, YH], F32, tag="ydiff")
            for h in range(2):
                nc.vector.tensor_scalar(
                    out=ydiff[:], in0=yramp[:, YH * h:YH * h + YH],
                    scalar1=pk[:, 48:49], scalar2=None, op0=AL.subtract)
                nc.vector.tensor_mul(sq[h][:], ydiff[:], ydiff[:])

            ebuf = epool.tile([128, 14 * NCH], F32, tag="ebuf")
            oxy = epool.tile([128, 2 * 2 * NCH], F16, tag="oxy")

            # ---- epilogue helpers: 2 passes over [128, 288] ----
            def V(s, e):
                return ebuf[:].rearrange(
                    "p (d k) -> p d k", k=14)[:, :, 7 * e + s:7 * e + s + 1]

            def dtile(tag):
                return tpool.tile([128, NCH], F32, tag=tag, name=tag)

            def r3(t):
                # dense [128, 288] viewed as [128, 288, 1] to match V() rank
                return t[:].rearrange("p (d k) -> p d k", k=1)

            # ---- main loop: 96 units, sums banks of 12 units ----
            for ub in range(NU // 12):
                sbank = pssum.tile([128, 504], F32, tag="sbank")
                for uu in range(12):
                    u = ub * 12 + uu
                    h = u % 2
                    icu = pk[:, u // 2:u // 2 + 1]
                    wt = wpool.tile([128, YH], F32, tag="wt")
                    # wt = 1/(sqy_h*icx_u + 1); true weight w = wt*icx_u, the
                    # icx_u factor rides in c2u below.
                    act_recip(nc, wt[:], sq[h][:], scale_ap=icu, bias=1.0)
                    c2u = upool.tile([128, 14], F32, tag="c2u")
                    nc.vector.tensor_scalar(out=c2u[:], in0=c2[:], scalar1=icu,
                                            scalar2=None, op0=AL.mult)
                    for c in range(3):
                        nc.tensor.matmul(
                            sbank[:, 14 * (uu * 3 + c):14 * (uu * 3 + c) + 14],
                            wt[:, 128 * c:128 * c + 128], c2u[:],
                            start=True, stop=True)
                nc.scalar.copy(out=ebuf[:, ub * 504:(ub + 1) * 504],
                               in_=sbank[:])

            for e in range(2):
                isw = dtile(f"isw{e}")
                nc.vector.reciprocal(out=r3(isw), in_=V(0, e))
                psx, psy = dtile(f"psx{e}"), dtile(f"psy{e}")
                qsx, qsy = dtile(f"qsx{e}"), dtile(f"qsy{e}")
                nc.vector.tensor_tensor(out=r3(psx), in0=V(1, e), in1=r3(isw), op=AL.mult)
                nc.vector.tensor_tensor(out=r3(psy), in0=V(2, e), in1=r3(isw), op=AL.mult)
                nc.vector.tensor_tensor(out=r3(qsx), in0=V(3, e), in1=r3(isw), op=AL.mult)
                nc.vector.tensor_tensor(out=r3(qsy), in0=V(4, e), in1=r3(isw), op=AL.mult)
                vpx, vpy = dtile(f"vpx{e}"), dtile(f"vpy{e}")
                nc.vector.tensor_sub(vpx[:], xg[e][:], psx[:])
                nc.vector.tensor_sub(vpy[:], yg[:], psy[:])
                a1, a2 = dtile(f"a1{e}"), dtile(f"a2{e}")
                nc.vector.tensor_tensor(out=r3(a1), in0=V(1, e), in1=V(3, e), op=AL.mult)
                nc.vector.tensor_tensor(out=r3(a2), in0=V(2, e), in1=V(4, e), op=AL.mult)
                nc.vector.tensor_add(a1[:], a1[:], a2[:])
                nc.vector.tensor_mul(a1[:], a1[:], isw[:])
                P = dtile(f"P{e}")
                nc.vector.tensor_tensor(out=r3(P), in0=V(5, e), in1=r3(a1), op=AL.subtract)
                b1, b2 = dtile(f"b1{e}"), dtile(f"b2{e}")
                nc.vector.tensor_tensor(out=r3(b1), in0=V(3, e), in1=V(2, e), op=AL.mult)
                nc.vector.tensor_tensor(out=r3(b2), in0=V(4, e), in1=V(1, e), op=AL.mult)
                nc.vector.tensor_sub(b1[:], b1[:], b2[:])
                nc.vector.tensor_mul(b1[:], b1[:], isw[:])
                Q = dtile(f"Q{e}")
                nc.vector.tensor_tensor(out=r3(Q), in0=V(6, e), in1=r3(b1), op=AL.subtract)
                fx1, fx2 = dtile(f"fx1{e}"), dtile(f"fx2{e}")
                nc.vector.tensor_mul(fx1[:], P[:], vpx[:])
                nc.vector.tensor_mul(fx2[:], Q[:], vpy[:])
                frvx = dtile(f"frvx{e}")
                nc.vector.tensor_add(frvx[:], fx1[:], fx2[:])
                nc.vector.tensor_mul(fx1[:], P[:], vpy[:])
                nc.vector.tensor_mul(fx2[:], Q[:], vpx[:])
                frvy = dtile(f"frvy{e}")
                nc.vector.tensor_sub(frvy[:], fx1[:], fx2[:])
                n1, n2 = dtile(f"n1{e}"), dtile(f"n2{e}")
                nc.vector.tensor_mul(n1[:], vpx[:], vpx[:])
                nc.vector.tensor_mul(n2[:], vpy[:], vpy[:])
                nc.vector.tensor_add(n1[:], n1[:], n2[:])
                nvp = dtile(f"nvp{e}")
                nc.scalar.sqrt(nvp[:], n1[:])
                nc.vector.tensor_mul(n1[:], frvx[:], frvx[:])
                nc.vector.tensor_mul(n2[:], frvy[:], frvy[:])
                nc.vector.tensor_add(n1[:], n1[:], n2[:])
                nfr = dtile(f"nfr{e}")
                nc.scalar.sqrt(nfr[:], n1[:])
                nc.vector.tensor_scalar(out=nfr[:], in0=nfr[:], scalar1=EPS_FRV,
                                        scalar2=0.0, op0=AL.add, op1=AL.add)
                rden = dtile(f"rden{e}")
                nc.vector.reciprocal(out=rden[:], in_=nfr[:])
                nc.vector.tensor_mul(rden[:], rden[:], nvp[:])   # scale
                nc.vector.tensor_mul(frvx[:], frvx[:], rden[:])
                nc.vector.tensor_mul(frvy[:], frvy[:], rden[:])
                # un-center qs: += CTR
                nc.vector.tensor_scalar(out=qsx[:], in0=qsx[:], scalar1=CTR,
                                        scalar2=0.0, op0=AL.add, op1=AL.add)
                nc.vector.tensor_scalar(out=qsy[:], in0=qsy[:], scalar1=CTR,
                                        scalar2=0.0, op0=AL.add, op1=AL.add)
                # final adds, h-split, writing interleaved out_xy (f16)
                # dense col d = u*3 + c = (2p+h)*3 + c ; fixed h:
                #   in dims (p: step 6, count 48), (c: step 1, count 3), off 3h
                # out col = (h*3+c)*192 + (2p+e)*2 + comp:
                #   out dims (p: step 4, count 48), (c: step 192, count 3),
                #   off 576h + 2e + comp
                for comp, (frv, qs) in enumerate(((frvx, qsx), (frvy, qsy))):
                    for h in range(2):
                        iv0 = frv[:].rearrange(
                            "p (pp x c) -> p pp x c", pp=48, x=2)[:, :, h, :]
                        iv1 = qs[:].rearrange(
                            "p (pp x c) -> p pp x c", pp=48, x=2)[:, :, h, :]
                        ov = oxy[:].rearrange(
                            "p (hh c pp t) -> p hh c pp t",
                            hh=2, c=3, pp=48)[:, h, :, :, 2 * e + comp]
                        ov = ov.rearrange("p c pp -> p pp c")
                        nc.vector.tensor_tensor(out=ov, in0=iv0, in1=iv1,
                                                op=AL.add)

            # ---- output DMA: per half, (x_loc, comp) contiguous runs ----
            for h in range(2):
                src = oxy[:].rearrange(
                    "p (hh c t) -> p hh c t", hh=2, c=3)[:, h, :, :]
                dst = outd[:].rearrange(
                    "(hh c p) t -> p hh c t", hh=2, c=3, p=128)[:, h, :, :]
                nc.sync.dma_start(out=dst, in_=src)

    # split >1-wait instructions (walrus codegen limit in this container)
    import concourse.mybir as mybir
    for f in nc.m.functions:
        for bb in f.blocks:
            newlist = []
            for inst in bb.instructions:
                si = inst.sync_info
                if si is not None and si.on_wait and len(si.on_wait) > 1:
                    waits = list(si.on_wait)
                    extra, keep = waits[:-1], waits[-1:]
                    for k, wchunk in enumerate(extra):
                        nop = mybir.InstNoOp(
                            name=f"{inst.name}-ws{k}", engine=inst.engine,
                            ins=[], outs=[],
                            sync_info=mybir.SyncInfo(on_wait=[wchunk],
                                                     on_update=[]))
                        newlist.append(nop)
                    inst.sync_info = mybir.SyncInfo(
                        on_wait=keep,
                        on_update=list(si.on_update) if si.on_update else [])
                newlist.append(inst)
            bb.instructions = newlist
    return nc


# Static per-input concat shapes, keyed by dram tensor name (order must match
# the ExternalInput declaration order in _build_nc).
_IN_SHAPES = {
    "pk": ((128, 63), np.float32),
    "yramp": ((128, H), np.float32),
    "xg0": ((128, NCH), np.float32),
    "xg1": ((128, NCH), np.float32),
    "yg": ((128, NCH), np.float32),
}


def _const_inputs():
    """pi/qi-independent device-resident inputs: concat [8*128, .] arrays."""
    u_of_d = np.arange(NCH) // 3
    c_of_d = np.arange(NCH) % 3
    p_of_d = u_of_d // 2
    h_of_d = u_of_d % 2
    r = np.arange(128)
    ygl = (YH * h_of_d[None, :] + 128 * c_of_d[None, :]
           + r[:, None]).astype(np.float64) - CTR
    yg1 = ygl.astype(np.float32)
    yramp = np.broadcast_to(
        np.arange(H, dtype=np.float32)[None, :], (128, H))
    xg0s, xg1s, ygs, yrs = [], [], [], []
    for core in range(NCORES):
        x0 = WLOC * core
        for e, dst in ((0, xg0s), (1, xg1s)):
            xv = (x0 + 2 * p_of_d + e).astype(np.float64) - CTR
            dst.append(np.broadcast_to(
                xv[None, :], (128, NCH)).astype(np.float32))
        ygs.append(yg1)
        yrs.append(yramp)
    return {"yramp": np.concatenate(yrs, 0),
            "xg0": np.concatenate(xg0s, 0), "xg1": np.concatenate(xg1s, 0),
            "yg": np.concatenate(ygs, 0)}


def _call_inputs(pi, qi):
    """pi/qi-dependent packed input, concatenated across the 8 cores."""
    pi = np.asarray(pi)
    qi = np.asarray(qi)
    pix, piy = pi[:, 0].astype(np.float64), pi[:, 1].astype(np.float64)
    qix, qiy = qi[:, 0].astype(np.float64), qi[:, 1].astype(np.float64)

    xs = np.arange(W, dtype=np.float64)
    ibx = 1.0 / ((xs[:, None] - pix[None, :]) ** 2 + EPS_D2)  # [768, 64]
    # The ACT Reciprocal table returns garbage for inputs beyond ~2e12, and
    # its input is sqy*icx + 1 <= 5.9e5*icx. Capping icx at 1e6 keeps that
    # below 6e11; the cap only binds within |x-pix| < 1e-3, where a 1e6
    # weight still dominates every other control point by >=4 orders.
    ibx32 = np.minimum(ibx, 1e6).astype(np.float32)

    # C2 [128, 14]: rows=points(parity blocks), cols 0:7 even-x sums,
    # 7:14 odd-x. Sum order: sw,Spx,Spy,Sqx,Sqy,Spq,Sx (centered coords).
    pxc, pyc = pix - CTR, piy - CTR
    qxc, qyc = qix - CTR, qiy - CTR
    cols = np.stack([np.ones(N), pxc, pyc, qxc, qyc,
                     pxc * qxc + pyc * qyc, qxc * pyc - qyc * pxc], 1)
    c2 = np.zeros((128, 14), np.float32)
    c2[:N, 0:7] = cols
    c2[N:, 7:14] = cols

    piyb = np.tile(pi[:, 1].astype(np.float32), 2)[:, None]  # [128, 1]

    p_ = np.arange(NU // 2)
    pks = []
    for core in range(NCORES):
        xe = WLOC * core + 2 * p_
        blk = np.empty((128, 63), np.float32)
        blk[:64, 0:48] = ibx32[xe, :].T
        blk[64:, 0:48] = ibx32[xe + 1, :].T
        blk[:, 48:49] = piyb
        blk[:, 49:63] = c2
        pks.append(blk)
    return {"pk": np.concatenate(pks, 0)}


def _get_exec():
    if "exec" in _CACHE:
        return _CACHE["exec"]
    import jax
    from jax.sharding import Mesh, PartitionSpec, NamedSharding
    from jax.experimental.shard_map import shard_map
    import concourse.bass2jax as b2j
    import concourse.mybir as mybir

    nc = _build_nc()
    b2j.install_neuronx_cc_hook()

    partition_name = (nc.partition_id_tensor.name
                      if nc.partition_id_tensor else None)
    in_names, out_names, out_avals = [], [], []
    for alloc in nc.m.functions[0].allocations:
        if not isinstance(alloc, mybir.MemoryLocationSet):
            continue
        name = alloc.memorylocations[0].name
        if alloc.kind == "ExternalInput":
            if name != partition_name:
                in_names.append(name)
        elif alloc.kind == "ExternalOutput":
            assert alloc.tensor_shape is not None and alloc.dtype is not None
            out_names.append(name)
            out_avals.append(jax.core.ShapedArray(
                tuple(alloc.tensor_shape), mybir.dt.np(alloc.dtype)))
    assert in_names == list(_IN_SHAPES), in_names
    all_in_names = tuple(in_names) + tuple(out_names)
    if partition_name is not None:
        all_in_names = all_in_names + (partition_name,)

    def _body(*args):
        operands = list(args)
        if partition_name is not None:
            operands.append(b2j.partition_id_tensor())
        outs = b2j._bass_exec_p.bind(
            *operands,
            out_avals=tuple(out_avals),
            in_names=all_in_names,
            out_names=tuple(out_names),
            lowering_input_output_aliases=(),
            sim_require_finite=True,
            sim_require_nnan=True,
            nc=nc,
        )
        return tuple(outs)

    devices = jax.devices()[:NCORES]
    assert len(devices) == NCORES, jax.devices()
    mesh = Mesh(np.asarray(devices), ("core",))
    Ps = PartitionSpec
    sh = NamedSharding(mesh, Ps("core"))
    n_params = len(in_names)
    n_outs = len(out_names)
    in_specs = (Ps("core"),) * (n_params + n_outs)
    out_specs = (Ps("core"),) * n_outs
    jitted = jax.jit(
        shard_map(_body, mesh=mesh, in_specs=in_specs,
                  out_specs=out_specs, check_rep=False),
        keep_unused=True,
    )
    avals = [
        jax.ShapeDtypeStruct((NCORES * s[0], *s[1:]), dt, sharding=sh)
        for (s, dt) in _IN_SHAPES.values()
    ] + [
        jax.ShapeDtypeStruct((NCORES * a.shape[0], *a.shape[1:]), a.dtype,
                             sharding=sh)
        for a in out_avals
    ]
    compiled = b2j.fast_dispatch_compile(
        lambda: jitted.lower(*avals).compile())

    consts = _const_inputs()
    const_dev = {k: jax.device_put(v, sh) for k, v in consts.items()}
    # The NEFF writes every element of "out"; this operand only exists so the
    # custom call signature matches — its contents are never read. A single
    # persistent device buffer serves all calls.
    out_dummy = jax.device_put(
        np.zeros((NCORES * out_avals[0].shape[0], *out_avals[0].shape[1:]),
                 out_avals[0].dtype), sh)

    ex = {
        "compiled": compiled, "sh": sh, "in_names": in_names,
        "const_dev": const_dev, "out_dummy": out_dummy, "jax": jax,
    }
    _CACHE["exec"] = ex
    return ex


def _kernel_once(pi, qi):
    ex = _get_exec()
    jax = ex["jax"]
    per_call = _call_inputs(np.asarray(pi), np.asarray(qi))
    args = [
        jax.device_put(per_call[name], ex["sh"]) if name in per_call
        else ex["const_dev"][name]
        for name in ex["in_names"]
    ]
    args.append(ex["out_dummy"])
    (out,) = ex["compiled"](*args)
    res = np.asarray(out)  # [8*768, 192] f16
    full = np.empty((H, W, 2), np.float32)
    r4 = res.reshape(NCORES, H, WLOC, 2)
    for c in range(NCORES):
        full[:, c * WLOC:(c + 1) * WLOC, :] = r4[c]
    return full


def kernel(img, pi, qi):
    try:
        return _kernel_once(pi, qi)
    except Exception:
        # A wedged core (NRT_EXEC_UNIT_UNRECOVERABLE) or dropped tunnel kills
        # the PJRT client; rebuild it once and retry before giving up.
        import time
        import jax
        _CACHE.clear()
        try:
            jax.clear_caches()
        except Exception:
            pass
        try:
            import jax.extend.backend as _jb
            _jb.clear_backends()
        except Exception:
            pass
        time.sleep(2.0)
        return _kernel_once(pi, qi)
